# revision 9
# baseline (speedup 1.0000x reference)
"""Trainium2 Bass kernel: 2D Haar DWT (single level) on x[8, 256, 256, 64] f32.

Math: with this problem's symmetric-pad + stride-2 slicing, the padding never
contributes; each output element is a +/- combination of one 2x2 spatial block:
    p = x[2i, 2j], q = x[2i, 2j+1], r = x[2i+1, 2j], s = x[2i+1, 2j+1]
    ll = 0.5(p+q+r+s)   -> out[0:128, 0:128]
    lh = 0.5(p+q-r-s)   -> out[128:256, 0:128]
    hl = 0.5(p-q+r-s)   -> out[0:128, 128:256]
    hh = 0.5(p-q-r+s)   -> out[128:256, 128:256]
(per channel; channels are the contiguous innermost dim)

Sharding: pure data-parallel, one batch example per NeuronCore (8 cores).

Per-core layout (v3 -- TensorE fp16 + ACT + DVE split):
  - partition dim = H row (two 128-row bands per image); free dim = W*C.
  - loads are SWDGE (gpsimd) DMAs that cast f32 DRAM -> fp16 SBUF inline.
    HBM read bytes unchanged; the fp16 rounding (~1e-4 rel) is far inside
    the 2e-2 correctness gate.
  - TensorE does the H-direction butterfly as out = BFLY.T @ X with a const
    128x128 fp16 matrix of +/-0.5 taps (so the 0.5 scale is free and fp16
    matmul avoids the fp32 LOW/HIGH double-pump that costs 4x PE time):
    PSUM partitions 0..63 = 0.5(row2j + row2j+1), 64..127 = 0.5(row2j - ...).
  - DVE can read only ONE operand from PSUM, so ACT evacuates each 4-bank
    PSUM chunk to SBUF in a single f32 copy; DVE then does the W-direction
    butterfly from SBUF (tensor_add/tensor_sub over even/odd W positions).
  - stores go on the now-otherwise-idle SP HWDGE ring, one DMA per output
    row-half so the DRAM AP keeps a 1D partition dim (a 2-level partition
    pattern makes HWDGE descriptor generation ~20x slower).
  Engine budget per core: PE ~25us, GpSimd(Q7) ~20us of SWDGE issue,
  ACT ~35us of copies, DVE ~35us, SP ~25us of store issue -- all far below
  the ~94us HBM roofline, so the DMA streams stay saturated.
"""

import numpy as np

import concourse.bacc as bacc
import concourse.mybir as mybir
from concourse import bass_utils
from concourse.tile import TileContext

B, H, W, C = 8, 256, 256, 64
ROW = W * C          # 16384 f32 per input row
# Load chunks: f32 elements per partition (per 128-row band). Small head
# chunks shrink the pipeline ramp; small tail chunks the final-store ramp.
CHUNKS = [512, 512, 1024, 2048, 2048, 2048, 2048, 2048, 2048, 1024, 1024]
assert sum(CHUNKS) == ROW
NFMAX = max(CHUNKS)
# Store groups (indices into CHUNKS): grouped so mid-kernel store DMA runs
# are 8KB (= the loads' 8KB HBM-read runs). The SDMA engines round-robin
# between the load and store queues at PACKET granularity, so equal run
# sizes give the two streams an equal share of HBM and they finish together.
SGROUPS = [[0], [1], [2], [3, 4], [5, 6], [7, 8], [9], [10]]
NGMAX = max(sum(CHUNKS[i] for i in g) for g in SGROUPS)

F32 = mybir.dt.float32
F16 = mybir.dt.float16
ADD = mybir.AluOpType.add
SUB = mybir.AluOpType.subtract


def _bfly_matrix() -> np.ndarray:
    """BFLY[k, j]: out[j, :] = sum_k BFLY[k, j] * in[k, :].
    j in 0..63: 0.5*(row 2j + row 2j+1); j in 64..127: 0.5*(row 2(j-64) - ...)."""
    Bm = np.zeros((128, 128), np.float16)
    i = np.arange(64)
    Bm[2 * i, i] = 0.5
    Bm[2 * i + 1, i] = 0.5
    Bm[2 * i, 64 + i] = 0.5
    Bm[2 * i + 1, 64 + i] = -0.5
    return Bm


def _dwt_tile_kernel(tc, out, x, bfly):
    nc = tc.nc
    # out rows = qh*128 + band*64 + i ; cols = qw*8192 + e  (e = out col in quad)
    outv = out.rearrange("(qh bb i) (qw e) -> qh i bb qw e", qh=2, bb=2, qw=2)

    with (
        tc.tile_pool(name="pw", bufs=1) as pw,
        tc.tile_pool(name="px", bufs=6) as px,
        tc.tile_pool(name="pm", bufs=4) as pm,
        tc.tile_pool(name="po", bufs=4) as po,
        tc.tile_pool(name="pp", bufs=2, space="PSUM") as pp,
    ):
        bf = pw.tile([128, 128], F16, name="bf")
        # const load on the ACT ring; loads ride the gpsimd (SWDGE) path
        nc.scalar.dma_start(out=bf, in_=bfly[:, :])

        for bb in range(2):
            offs = [sum(CHUNKS[:i]) for i in range(len(CHUNKS))]
            for gi, grp in enumerate(SGROUPS):
                NG = sum(CHUNKS[i] for i in grp)
                goff = offs[grp[0]]
                ot = po.tile([128, NG], F32, name=f"ot{bb}_{gi}", tag="ot",
                             padded_shape=[128, NGMAX])
                og = ot.rearrange("p (g e) -> p g e", g=2)
                loc = 0
                for ci in grp:
                    NF, off = CHUNKS[ci], offs[ci]
                    xt = px.tile([128, NF], F16, name=f"xt{bb}_{ci}", tag="xt",
                                 padded_shape=[128, NFMAX])
                    # SWDGE load with inline f32 -> fp16 cast
                    nc.gpsimd.dma_start(
                        out=xt, in_=x[128 * bb : 128 * (bb + 1), off : off + NF]
                    )
                    pt = pp.tile([128, NF], F32, name=f"ps{bb}_{ci}", tag="ps",
                                 padded_shape=[128, NFMAX])
                    for m in range(NF // 512):
                        # one matmul per PSUM bank (512 f32 output limit)
                        nc.tensor.matmul(
                            pt[:, 512 * m : 512 * (m + 1)], bf,
                            xt[:, 512 * m : 512 * (m + 1)], start=True, stop=True,
                        )
                    # ACT evacuates the whole chunk PSUM->SBUF in one copy
                    md = pm.tile([128, NF], F32, name=f"md{bb}_{ci}", tag="md",
                                 padded_shape=[128, NFMAX])
                    nc.scalar.copy(md, pt)
                    # DVE: W-direction butterfly (even/odd W positions)
                    mv = md.rearrange("p (jl dj c) -> p jl dj c", dj=2, c=C)
                    nc.vector.tensor_add(
                        out=og[:, 0, loc : loc + NF // 2].rearrange(
                            "p (jl c) -> p jl c", c=C),
                        in0=mv[:, :, 0, :], in1=mv[:, :, 1, :],
                    )
                    nc.vector.tensor_sub(
                        out=og[:, 1, loc : loc + NF // 2].rearrange(
                            "p (jl c) -> p jl c", c=C),
                        in0=mv[:, :, 0, :], in1=mv[:, :, 1, :],
                    )
                    loc += NF // 2
                # two stores (one per output row-half) on the SP ring; each has
                # a clean 1D partition dim. partitions 0..63 -> ll|hl rows,
                # 64..127 -> lh|hh rows; free halves [W-sums | W-diffs] -> qw
                otv = ot.rearrange("(qh i) (qw e) -> qh i qw e", qh=2, qw=2)
                for qh in range(2):
                    nc.sync.dma_start(
                        out=outv[qh, :, bb, :, goff // 2 : (goff + NG) // 2],
                        in_=otv[qh],
                    )


_NC_CACHE = None


def _get_nc():
    global _NC_CACHE
    if _NC_CACHE is None:
        nc = bacc.Bacc("TRN2", target_bir_lowering=False, debug=False)
        x = nc.dram_tensor("x", [H, ROW], F32, kind="ExternalInput").ap()
        out = nc.dram_tensor("out", [H, ROW], F32, kind="ExternalOutput").ap()
        bfly = nc.inline_tensor(_bfly_matrix(), name="bfly").ap()
        with TileContext(nc) as tc:
            _dwt_tile_kernel(tc, out, x, bfly)
        nc.compile()
        _NC_CACHE = nc
    return _NC_CACHE


def kernel(x: np.ndarray) -> np.ndarray:
    assert x.shape == (B, H, W, C), x.shape
    nc = _get_nc()
    in_maps = [
        {"x": np.ascontiguousarray(x[b], dtype=np.float32).reshape(H, ROW)}
        for b in range(B)
    ]
    res = bass_utils.run_bass_kernel_spmd(nc, in_maps, core_ids=list(range(B)))
    return np.stack(
        [r["out"].reshape(H, W, C) for r in res.results], axis=0
    ).astype(x.dtype, copy=False)


# revision 11
# speedup vs baseline: 1.1817x; 1.1817x over previous
"""Trainium2 Bass kernel: 2D Haar DWT (single level) on x[8, 256, 256, 64] f32.

Math: with this problem's symmetric-pad + stride-2 slicing, the padding never
contributes; each output element is a +/- combination of one 2x2 spatial block:
    p = x[2i, 2j], q = x[2i, 2j+1], r = x[2i+1, 2j], s = x[2i+1, 2j+1]
    ll = 0.5(p+q+r+s)   -> out[0:128, 0:128]
    lh = 0.5(p+q-r-s)   -> out[128:256, 0:128]
    hl = 0.5(p-q+r-s)   -> out[0:128, 128:256]
    hh = 0.5(p-q-r+s)   -> out[128:256, 128:256]
(per channel; channels are the contiguous innermost dim)

Sharding: pure data-parallel, one batch example per NeuronCore (8 cores).

Per-core layout (v3 -- TensorE fp16 + ACT + DVE split):
  - partition dim = H row (two 128-row bands per image); free dim = W*C.
  - loads are SWDGE (gpsimd) DMAs that cast f32 DRAM -> fp16 SBUF inline.
    HBM read bytes unchanged; the fp16 rounding (~1e-4 rel) is far inside
    the 2e-2 correctness gate.
  - TensorE does the H-direction butterfly as out = BFLY.T @ X with a const
    128x128 fp16 matrix of +/-0.5 taps (so the 0.5 scale is free and fp16
    matmul avoids the fp32 LOW/HIGH double-pump that costs 4x PE time):
    PSUM partitions 0..63 = 0.5(row2j + row2j+1), 64..127 = 0.5(row2j - ...).
  - DVE can read only ONE operand from PSUM, so ACT evacuates each 4-bank
    PSUM chunk to SBUF in a single f32 copy; DVE then does the W-direction
    butterfly from SBUF (tensor_add/tensor_sub over even/odd W positions).
  - stores go on the now-otherwise-idle SP HWDGE ring, one DMA per output
    row-half so the DRAM AP keeps a 1D partition dim (a 2-level partition
    pattern makes HWDGE descriptor generation ~20x slower).
  Engine budget per core: PE ~25us, GpSimd(Q7) ~20us of SWDGE issue,
  ACT ~35us of copies, DVE ~35us, SP ~25us of store issue -- all far below
  the ~94us HBM roofline, so the DMA streams stay saturated.
"""

import numpy as np

import concourse.bacc as bacc
import concourse.mybir as mybir
from concourse import bass_utils
from concourse.tile import TileContext

B, H, W, C = 8, 256, 256, 64
ROW = W * C          # 16384 f32 per input row
# Load chunks: f32 elements per partition (per 128-row band). Small head
# chunks shrink the pipeline ramp; small tail chunks the final-store ramp.
CHUNKS = [1024, 1024, 2048, 2048, 2048, 2048, 2048, 2048, 1024, 1024]
assert sum(CHUNKS) == ROW
NFMAX = max(CHUNKS)
# One store per load chunk (grouping stores into bigger DMAs was tried and
# made the pipeline lumpier: loads stalled on the deeper ot dependency).
SGROUPS = [[i] for i in range(len(CHUNKS))]
NGMAX = max(sum(CHUNKS[i] for i in g) for g in SGROUPS)

F32 = mybir.dt.float32
F16 = mybir.dt.float16
ADD = mybir.AluOpType.add
SUB = mybir.AluOpType.subtract


def _bfly_matrix() -> np.ndarray:
    """BFLY[k, j]: out[j, :] = sum_k BFLY[k, j] * in[k, :].
    j in 0..63: 0.5*(row 2j + row 2j+1); j in 64..127: 0.5*(row 2(j-64) - ...)."""
    Bm = np.zeros((128, 128), np.float16)
    i = np.arange(64)
    Bm[2 * i, i] = 0.5
    Bm[2 * i + 1, i] = 0.5
    Bm[2 * i, 64 + i] = 0.5
    Bm[2 * i + 1, 64 + i] = -0.5
    return Bm


def _dwt_tile_kernel(tc, out, x, bfly):
    nc = tc.nc
    # out rows = qh*128 + band*64 + i ; cols = qw*8192 + e  (e = out col in quad)
    outv = out.rearrange("(qh bb i) (qw e) -> qh i bb qw e", qh=2, bb=2, qw=2)

    with (
        tc.tile_pool(name="pw", bufs=1) as pw,
        tc.tile_pool(name="px", bufs=8) as px,
        tc.tile_pool(name="pm", bufs=4) as pm,
        tc.tile_pool(name="po", bufs=6) as po,
        tc.tile_pool(name="pp", bufs=2, space="PSUM") as pp,
    ):
        bf = pw.tile([128, 128], F16, name="bf")
        # const load on the ACT ring; loads ride the gpsimd (SWDGE) path
        nc.scalar.dma_start(out=bf, in_=bfly[:, :])

        for bb in range(2):
            offs = [sum(CHUNKS[:i]) for i in range(len(CHUNKS))]
            for gi, grp in enumerate(SGROUPS):
                NG = sum(CHUNKS[i] for i in grp)
                goff = offs[grp[0]]
                ot = po.tile([128, NG], F32, name=f"ot{bb}_{gi}", tag="ot",
                             padded_shape=[128, NGMAX])
                og = ot.rearrange("p (g e) -> p g e", g=2)
                loc = 0
                for ci in grp:
                    NF, off = CHUNKS[ci], offs[ci]
                    xt = px.tile([128, NF], F16, name=f"xt{bb}_{ci}", tag="xt",
                                 padded_shape=[128, NFMAX])
                    # SWDGE load with inline f32 -> fp16 cast
                    nc.gpsimd.dma_start(
                        out=xt, in_=x[128 * bb : 128 * (bb + 1), off : off + NF]
                    )
                    pt = pp.tile([128, NF], F32, name=f"ps{bb}_{ci}", tag="ps",
                                 padded_shape=[128, NFMAX])
                    for m in range(NF // 512):
                        # one matmul per PSUM bank (512 f32 output limit)
                        nc.tensor.matmul(
                            pt[:, 512 * m : 512 * (m + 1)], bf,
                            xt[:, 512 * m : 512 * (m + 1)], start=True, stop=True,
                        )
                    # ACT evacuates the whole chunk PSUM->SBUF in one copy
                    md = pm.tile([128, NF], F32, name=f"md{bb}_{ci}", tag="md",
                                 padded_shape=[128, NFMAX])
                    nc.scalar.copy(md, pt)
                    # DVE: W-direction butterfly (even/odd W positions)
                    mv = md.rearrange("p (jl dj c) -> p jl dj c", dj=2, c=C)
                    nc.vector.tensor_add(
                        out=og[:, 0, loc : loc + NF // 2].rearrange(
                            "p (jl c) -> p jl c", c=C),
                        in0=mv[:, :, 0, :], in1=mv[:, :, 1, :],
                    )
                    nc.vector.tensor_sub(
                        out=og[:, 1, loc : loc + NF // 2].rearrange(
                            "p (jl c) -> p jl c", c=C),
                        in0=mv[:, :, 0, :], in1=mv[:, :, 1, :],
                    )
                    loc += NF // 2
                # two stores (one per output row-half) on the SP ring; each has
                # a clean 1D partition dim. partitions 0..63 -> ll|hl rows,
                # 64..127 -> lh|hh rows; free halves [W-sums | W-diffs] -> qw
                otv = ot.rearrange("(qh i) (qw e) -> qh i qw e", qh=2, qw=2)
                for qh in range(2):
                    nc.sync.dma_start(
                        out=outv[qh, :, bb, :, goff // 2 : (goff + NG) // 2],
                        in_=otv[qh],
                    )


_NC_CACHE = None


def _get_nc():
    global _NC_CACHE
    if _NC_CACHE is None:
        nc = bacc.Bacc("TRN2", target_bir_lowering=False, debug=False)
        x = nc.dram_tensor("x", [H, ROW], F32, kind="ExternalInput").ap()
        out = nc.dram_tensor("out", [H, ROW], F32, kind="ExternalOutput").ap()
        bfly = nc.inline_tensor(_bfly_matrix(), name="bfly").ap()
        with TileContext(nc) as tc:
            _dwt_tile_kernel(tc, out, x, bfly)
        nc.compile()
        _NC_CACHE = nc
    return _NC_CACHE


def kernel(x: np.ndarray) -> np.ndarray:
    assert x.shape == (B, H, W, C), x.shape
    nc = _get_nc()
    in_maps = [
        {"x": np.ascontiguousarray(x[b], dtype=np.float32).reshape(H, ROW)}
        for b in range(B)
    ]
    res = bass_utils.run_bass_kernel_spmd(nc, in_maps, core_ids=list(range(B)))
    return np.stack(
        [r["out"].reshape(H, W, C) for r in res.results], axis=0
    ).astype(x.dtype, copy=False)
